# revision 2
# baseline (speedup 1.0000x reference)
"""Trainium2 Bass kernel for nn_KalmanFilter: EKF over T=512 steps, N=8192 chains.

Mathematical reduction (verified exact vs the reference):
  With C = [[0,0,0,1,0],[0,0,0,0,1]], rows 3,4 of the Jacobian A are zero, so
  columns 3,4 of Sigma_pred are exactly e3,e4 and S = I + R depends only on the
  per-step measurement parameters. The covariance never influences the output.
  The computation collapses to, per chain:
    S = I + L L^T,  L = [[e^l0, 0], [l1, e^l2]]
    u_{t+1} = (I - S^-1) u_t + S^-1 z_t          (u = [v, omega])
    th_{t+1} = th_t + omega_t * dt_t
    x_{t+1}  = x_t + v_t * dt_t * cos(th_t)
    y_{t+1}  = y_t + v_t * dt_t * sin(th_t)
    y_hat[t] = [x_{t+1}, y_{t+1}, th_{t+1}]
  The coupled 2-state linear recurrence is solved with Gauss-Seidel sweeps of
  hardware affine scans (tensor_tensor_scan); error contracts ~10x per sweep,
  SWEEPS=4 gives ~1e-4 absolute output error (fp32 floor is ~1.4e-5).

Sharding: data-parallel over chains, 1024 chains per NeuronCore across 8 cores.
"""
import sys
sys.path.insert(0, '/opt/trn_rl_repo')
import numpy as np
import concourse.bass as bass
from concourse import mybir
from concourse.bass_utils import run_bass_kernel_spmd

F32 = mybir.dt.float32
AF = mybir.ActivationFunctionType
A = mybir.AluOpType

N_CORES = 8
T = 512
N_TOT = 8192
NPC = N_TOT // N_CORES          # 1024 chains per core
P = 128                         # partitions
NSL = 4                         # slabs per core
CH = NPC // (NSL * P)           # chains per partition per slab = 2
SWEEPS = 4
MAGIC = float(1.5 * 2 ** 23)    # fp32 round-to-nearest trick
TWO_PI = float(2 * np.pi)
HALF_PI = float(np.pi / 2)


class _Sched:
    """Two-phase scheduler: record ops (engine, emit closure, deps), then emit
    per-engine in-order streams. Cross-engine deps become standalone wait_ge
    instructions (this walrus supports only one wait condition per inst)."""

    def __init__(self):
        self.ops = []
        self.count = {"v": 0, "g": 0, "a": 0, "s": 0}

    def __init_slots(self):
        pass

    def add(self, eng, emit_fn, deps=(), slot=None):
        self.count[eng] += 1
        ref = (eng, self.count[eng])
        if eng == "s":
            if not hasattr(self, "slot_count"):
                self.slot_count = {}
            self.slot_count[slot] = self.slot_count.get(slot, 0) + 1
            ref = ("D", slot, self.slot_count[slot])
        self.ops.append((eng, emit_fn, tuple(d for d in deps if d), ref))
        return ref

    def emit(self, eng, raw_eng, sems, dma_sems):
        last = {}
        dlast = {}
        for op_eng, emit_fn, deps, ref in self.ops:
            if op_eng != eng:
                continue
            for dep in deps:
                if dep[0] == "D":
                    _, slot, k = dep
                    if dlast.get(slot, 0) >= k:
                        continue
                    raw_eng.wait_ge(dma_sems[slot], 16 * k)
                    dlast[slot] = k
                else:
                    deng, dpos = dep
                    if deng == eng or last.get(deng, 0) >= dpos:
                        continue
                    raw_eng.wait_ge(sems[deng], dpos)
                    last[deng] = dpos
            emit_fn().then_inc(sems[eng], 1)


def _build_nc(reps=1):
    nc = bass.Bass()
    IN = nc.dram_tensor("inp", [NSL, 6, P, CH, T], F32, kind="ExternalInput")
    MU = nc.dram_tensor("mu", [NSL, 5, P, CH], F32, kind="ExternalInput")
    OUT = nc.dram_tensor("out", [NSL, 3, P, CH, T], F32, kind="ExternalOutput")

    _names = [0]

    def tile(shape):
        _names[0] += 1
        return nc.alloc_sbuf_tensor(f"tl{_names[0]}", list(shape), F32).ap()

    if True:
        consts = tile([P, T + 4])
        ones = consts[:, 0:T]
        halfpi = consts[:, T:T + 1]
        zin = [tile([P, 6, CH, T]) for _ in range(2)]
        mu_all = tile([P, 2, 5, CH])
        mu = [mu_all[:, 0], mu_all[:, 1]]
        e0sq = tile([P, CH, T]); e2sq = tile([P, CH, T])
        e0 = tile([P, CH, T]);   l1sq = tile([P, CH, T])
        t1 = tile([P, CH, T])
        t2 = tile([P, CH, T]);   d1 = tile([P, CH, T])
        det = tile([P, CH, T])
        lnd = d1                              # alias: d1 dead before Ln
        r = tile([P, CH, T])
        s01 = e0                              # alias: written in place
        m01 = tile([P, CH, T])
        s11 = tile([P, CH, T]);  u0 = tile([P, CH, T]);  u1 = tile([P, CH, T])
        m00 = tile([P, CH, T]);  m11 = tile([P, CH, T])
        p0 = tile([P, CH, T]);   q0 = tile([P, CH, T]);  b0 = tile([P, CH, T])
        p1 = e2sq                             # alias: e2sq dead after s11
        q1 = tile([P, CH, T]);   b1 = tile([P, CH, T])
        dt = tile([P, CH, T])
        big = tile([P, 12, CH, T + 1])
        v = big[:, 0]; w = big[:, 1]; th = big[:, 2]; thr = big[:, 3]
        sinf = big[:, 4]; cosf = big[:, 5]
        x = [big[:, 6], big[:, 7]]
        y = [big[:, 8], big[:, 9]]
        tho = [big[:, 10], big[:, 11]]
        kf = thr                              # in-place range reduction
        cv = tile([P, CH, T]);  cw = tile([P, CH, T])
        vdt = tile([P, CH, T])
        gx = tile([P, CH, T])
        gy = tile([P, CH, T])

        sch = _Sched()
        # `pv` holds previous slab's refs for write-after-read protection.
        pv = {}
        out_done = {}   # (rep, s) -> ("d", thr) after that slab's out-DMAs
        z_done = {}     # (rep, s) -> last reader of zin/mu buffer of slab s

        c_ones = sch.add("v", lambda: nc.vector.memset(ones, 1.0))
        c_hpi = sch.add("v", lambda: nc.vector.memset(halfpi, HALF_PI))

        S = {}          # global slab idx -> dict of refs

        def stage_dma_in(G):
            g = G % (NSL)
            s = g
            bi = s % 2
            z = zin[bi]; m_ = mu[bi]
            prev = S.get(G - 2, {})
            d_in = sch.add("s", lambda z=z, s=s: nc.sync.dma_start(
                z[:], IN[s].rearrange("k p c t -> p k c t")),
                deps=(prev.get("ysc"),), slot=s * 5 + 0)
            d_mu = sch.add("s", lambda m_=m_, s=s: nc.sync.dma_start(
                m_[:], MU[s].rearrange("k p c -> p k c")),
                deps=(prev.get("ysc"),), slot=s * 5 + 1)
            S.setdefault(G, {}).update(din=d_in, dmu=d_mu)

        def stage_act_leaf(G):
            s = G % NSL; bi = s % 2
            z = zin[bi]
            l0 = z[:, 2]; l1 = z[:, 3]; l2 = z[:, 4]
            C = S[G]; P1 = S.get(G - 1, {})
            C["a1"] = sch.add("a", lambda l0=l0: nc.scalar.activation(
                e0sq[:], l0, AF.Exp, scale=2.0), deps=(C["din"], P1.get("v1b")))
            C["a2"] = sch.add("a", lambda l2=l2: nc.scalar.activation(
                e2sq[:], l2, AF.Exp, scale=2.0), deps=(P1.get("v9"),))
            C["a3"] = sch.add("a", lambda l0=l0: nc.scalar.activation(
                e0[:], l0, AF.Exp), deps=(P1.get("v3"),))
            C["a4"] = sch.add("a", lambda l1=l1: nc.scalar.activation(
                l1sq[:], l1, AF.Square), deps=(P1.get("v2"), P1.get("g3")))

        def stage_leaffront(G):
            C = S[G]; P1 = S.get(G - 1, {})
            C["v1"] = sch.add("v", lambda: nc.vector.tensor_scalar(
                t2[:], e2sq[:], 1.0, None, op0=A.add), deps=(C["a2"], P1.get("g3")))
            C["v1b"] = sch.add("v", lambda: nc.vector.tensor_scalar(
                t1[:], e0sq[:], 1.0, None, op0=A.add), deps=(C["a1"], P1.get("g4")))
            C["g1"] = sch.add("g", lambda: nc.gpsimd.tensor_tensor(
                d1[:], t1[:], t2[:], A.mult), deps=(C["v1b"], C["v1"], P1.get("v2")))
            C["v2"] = sch.add("v", lambda: nc.vector.tensor_tensor(
                det[:], d1[:], l1sq[:], A.add), deps=(C["g1"], C["a4"], P1.get("a5")))
            C["a5"] = sch.add("a", lambda: nc.scalar.activation(
                lnd[:], det[:], AF.Ln), deps=(C["v2"],))
            C["a6"] = sch.add("a", lambda: nc.scalar.activation(
                r[:], lnd[:], AF.Exp, scale=-1.0),
                deps=(C["a5"], P1.get("g4"), P1.get("v4")))

        def stage_act_trig(G):
            if G < 0:
                return
            C = S[G]; P1 = S.get(G - 1, {})
            C["asin"] = sch.add("a", lambda: nc.scalar.activation(
                sinf[:], thr[:], AF.Sin), deps=(C["k3"], P1.get("ggy")))
            C["acos"] = sch.add("a", lambda: nc.scalar.activation(
                cosf[:], thr[:], AF.Sin, scale=0.5),
                deps=(C["k3"], P1.get("ggx"), P1.get("vcos")))
            C["acos2"] = sch.add("a", lambda: nc.scalar.activation(
                cosf[:], cosf[:], AF.Square))

        def stage_leafback(G):
            s = G % NSL; bi = s % 2
            z = zin[bi]
            z0 = z[:, 0]; z1 = z[:, 1]; l1 = z[:, 3]
            times = z[:, 5]
            C = S[G]; P1 = S.get(G - 1, {})
            C["g2"] = sch.add("g", lambda: nc.gpsimd.tensor_tensor(
                s01[:], e0[:], l1[:], A.mult), deps=(C["a3"], C["din"], P1.get("v3")))
            C["v3"] = sch.add("v", lambda: nc.vector.tensor_tensor(
                m01[:], s01[:], r[:], A.mult), deps=(C["g2"], C["a6"], P1.get("g7")))
            C["g3"] = sch.add("g", lambda: nc.gpsimd.tensor_tensor(
                s11[:], t2[:], l1sq[:], A.add), deps=(C["a4"], C["v1"], P1.get("v4")))
            C["v4"] = sch.add("v", lambda: nc.vector.tensor_tensor(
                u0[:], s11[:], r[:], A.mult), deps=(C["g3"], C["a6"], P1.get("g5")))
            C["g4"] = sch.add("g", lambda: nc.gpsimd.tensor_tensor(
                u1[:], t1[:], r[:], A.mult), deps=(C["v1b"], C["a6"], P1.get("v8")))
            C["v5"] = sch.add("v", lambda: nc.vector.tensor_scalar(
                m00[:], u0[:], -1.0, 1.0, op0=A.mult, op1=A.add), deps=(C["v4"],))
            C["v6"] = sch.add("v", lambda: nc.vector.tensor_scalar(
                m11[:], u1[:], -1.0, 1.0, op0=A.mult, op1=A.add), deps=(C["g4"],))
            C["g5"] = sch.add("g", lambda: nc.gpsimd.tensor_tensor(
                p0[:], u0[:], z0[:], A.mult), deps=(C["v4"],))
            C["v7"] = sch.add("v", lambda: nc.vector.tensor_tensor(
                q0[:], m01[:], z1[:], A.mult), deps=(C["v3"], P1.get("g6")))
            C["g6"] = sch.add("g", lambda: nc.gpsimd.tensor_tensor(
                b0[:], p0[:], q0[:], A.subtract),
                deps=(C["g5"], C["v7"], P1.get("addv_last")))
            C["v8"] = sch.add("v", lambda: nc.vector.tensor_tensor(
                p1[:], u1[:], z1[:], A.mult), deps=(C["g4"],))
            C["g7"] = sch.add("g", lambda: nc.gpsimd.tensor_tensor(
                q1[:], m01[:], z0[:], A.mult), deps=(C["v3"],))
            C["v9"] = sch.add("v", lambda: nc.vector.tensor_tensor(
                b1[:], p1[:], q1[:], A.subtract), deps=(C["v8"], C["g7"]))

        def stage_down2(G):
            if G < 0:
                return
            s = G % NSL; bi = s % 2
            m_ = mu[bi]; xo = x[bi]; yo = y[bi]; tho_ = tho[bi]
            C = S[G]; P1 = S.get(G - 1, {}); P2 = S.get(G - 2, {})
            C["vcos"] = sch.add("v", lambda: nc.vector.tensor_scalar(
                cosf[:], cosf[:], -2.0, 1.0, op0=A.mult, op1=A.add),
                deps=(C["acos2"],))
            C["gvdt"] = sch.add("g", lambda: nc.gpsimd.tensor_tensor(
                vdt[:], v[:, :, 0:T], dt[:], A.mult),
                deps=(C["lastv"], C["g9"], C["i1"], P1.get("ggx")))
            C["ggx"] = sch.add("v", lambda: nc.vector.tensor_tensor(
                gx[:], vdt[:], cosf[:, :, 0:T], A.mult),
                deps=(C["gvdt"], C["vcos"]))
            C["ggy"] = sch.add("g", lambda: nc.gpsimd.tensor_tensor(
                gy[:], vdt[:], sinf[:, :, 0:T], A.mult),
                deps=(C["gvdt"], C["asin"], C["thsc"]))
            xs = []
            for c in range(CH):
                xs.append(sch.add("v", lambda c=c, xo=xo, m_=m_: nc.vector.tensor_tensor_scan(
                    xo[:, c, 1:T + 1], ones, gx[:, c], m_[:, 0, c:c + 1],
                    A.mult, A.add), deps=(C["ggx"], C["i4"], c_ones)))
            ys = []
            for c in range(CH):
                ys.append(sch.add("v", lambda c=c, yo=yo, m_=m_: nc.vector.tensor_tensor_scan(
                    yo[:, c, 1:T + 1], ones, gy[:, c], m_[:, 1, c:c + 1],
                    A.mult, A.add), deps=(C["ggy"], C["i5"], c_ones)))
            C["xsc"] = xs[-1]; C["ysc"] = ys[-1]
            C["thcopy"] = sch.add("g", lambda tho_=tho_: nc.gpsimd.tensor_copy(
                tho_[:], th[:]),
                deps=(C["thsc"], C["i3"], S.get(G - 2, {}).get("ot")))
            s5 = (G % NSL) * 5
            C["ox"] = sch.add("s", lambda xo=xo, s=s: nc.sync.dma_start(
                OUT[s, 0], xo[:, :, 1:T + 1]), deps=(C["xsc"],), slot=s5 + 2)
            C["oy"] = sch.add("s", lambda yo=yo, s=s: nc.sync.dma_start(
                OUT[s, 1], yo[:, :, 1:T + 1]), deps=(C["ysc"],), slot=s5 + 3)
            C["ot"] = sch.add("s", lambda tho_=tho_, s=s: nc.sync.dma_start(
                OUT[s, 2], tho_[:, :, 1:T + 1]), deps=(C["thcopy"],), slot=s5 + 4)

        def stage_inits(G):
            s = G % NSL; bi = s % 2
            m_ = mu[bi]; xo = x[bi]; yo = y[bi]
            C = S[G]; P1 = S.get(G - 1, {}); P2 = S.get(G - 2, {})
            C["i1"] = sch.add("v", lambda m_=m_: nc.vector.tensor_copy(
                v[:, :, 0], m_[:, 3]), deps=(C["din"], C["dmu"], P1.get("gvdt")))
            C["i2"] = sch.add("v", lambda m_=m_: nc.vector.tensor_copy(
                w[:, :, 0], m_[:, 4]), deps=(P1.get("gth"),))
            C["i3"] = sch.add("v", lambda m_=m_: nc.vector.tensor_copy(
                th[:, :, 0], m_[:, 2]), deps=(P1.get("thcopy"),))
            C["i4"] = sch.add("v", lambda m_=m_, xo=xo: nc.vector.tensor_copy(
                xo[:, :, 0], m_[:, 0]), deps=(P2.get("ox"),))
            C["i5"] = sch.add("v", lambda m_=m_, yo=yo: nc.vector.tensor_copy(
                yo[:, :, 0], m_[:, 1]), deps=(P2.get("oy"),))

        def stage_sweeps(G):
            s = G % NSL; bi = s % 2
            m_ = mu[bi]
            C = S[G]; P1 = S.get(G - 1, {})
            last_v = None; last_w = None; addv = None
            for k in range(SWEEPS):
                if k == 0:
                    dv = b0; dep_in = (C["g6"],)
                else:
                    mulv = sch.add("v", lambda: nc.vector.tensor_tensor(
                        cv[:], m01[:], w[:, :, 0:T], A.mult),
                        deps=(last_w, C["v3"], C["i2"]))
                    addv = sch.add("v", lambda: nc.vector.tensor_tensor(
                        cv[:], cv[:], b0[:], A.add), deps=(mulv, C["g6"]))
                    dv = cv; dep_in = (addv,)
                vs = []
                for c in range(CH):
                    vs.append(sch.add("v", lambda c=c, dv=dv, m_=m_: nc.vector.tensor_tensor_scan(
                        v[:, c, 1:T + 1], m00[:, c], dv[:, c], m_[:, 3, c:c + 1],
                        A.mult, A.add), deps=dep_in + (C["v5"], C["i1"], P1.get("gvdt"))))
                last_v = vs[-1]
                mulw = sch.add("v", lambda: nc.vector.tensor_tensor(
                    cw[:], m01[:], v[:, :, 0:T], A.mult), deps=(last_v, C["v3"]))
                addw = sch.add("v", lambda: nc.vector.tensor_tensor(
                    cw[:], cw[:], b1[:], A.add), deps=(mulw, C["v9"]))
                ws = []
                for c in range(CH):
                    ws.append(sch.add("v", lambda c=c, m_=m_: nc.vector.tensor_tensor_scan(
                        w[:, c, 1:T + 1], m11[:, c], cw[:, c], m_[:, 4, c:c + 1],
                        A.mult, A.add), deps=(addw, C["v6"], C["i2"], P1.get("gth"))))
                last_w = ws[-1]
            C["lastv"] = last_v; C["lastw"] = last_w; C["addv_last"] = addv

        def stage_down1(G):
            s = G % NSL; bi = s % 2
            z = zin[bi]; m_ = mu[bi]
            times = z[:, 5]
            C = S[G]; P1 = S.get(G - 1, {})
            C["g8"] = sch.add("g", lambda times=times: nc.gpsimd.tensor_tensor(
                dt[:, :, 1:T], times[:, :, 1:T], times[:, :, 0:T - 1], A.subtract),
                deps=(C["din"], P1.get("gvdt")))
            C["g9"] = sch.add("g", lambda: nc.gpsimd.memset(dt[:, :, 0], 0.0))
            C["gth"] = sch.add("g", lambda: nc.gpsimd.tensor_tensor(
                gy[:], w[:, :, 0:T], dt[:], A.mult),
                deps=(C["lastw"], C["g9"], C["i2"], P1.get("ysc")))
            ths = []
            for c in range(CH):
                ths.append(sch.add("v", lambda c=c, m_=m_: nc.vector.tensor_tensor_scan(
                    th[:, c, 1:T + 1], ones, gy[:, c], m_[:, 2, c:c + 1],
                    A.mult, A.add), deps=(C["gth"], c_ones, C["i3"])))
            C["thsc"] = ths[-1]
            k1 = sch.add("v", lambda: nc.vector.tensor_scalar(
                kf[:], th[:], 1.0 / TWO_PI, MAGIC, op0=A.mult, op1=A.add),
                deps=(C["thsc"], P1.get("acos2")))
            k2 = sch.add("v", lambda: nc.vector.tensor_scalar(
                kf[:], kf[:], MAGIC, None, op0=A.subtract), deps=(k1,))
            C["k3"] = sch.add("v", lambda: nc.vector.scalar_tensor_tensor(
                thr[:], kf[:], -TWO_PI, th[:], A.mult, A.add), deps=(k2,))

        NG = reps * NSL
        for G in range(NG):
            stage_dma_in(G)
            stage_act_leaf(G)
            stage_leaffront(G)
            stage_act_trig(G - 1)
            stage_leafback(G)
            stage_down2(G - 1)
            stage_inits(G)
            stage_sweeps(G)
            stage_down1(G)
        stage_act_trig(NG - 1)
        stage_down2(NG - 1)

        n_slots = NSL * 5
        sem_v = nc.alloc_semaphore()
        sem_g = nc.alloc_semaphore()
        sem_a = nc.alloc_semaphore()
        dma_sems = [nc.alloc_semaphore(f"dsem{i}") for i in range(n_slots)]
        with nc.Block() as block:
            sems = {"v": sem_v, "g": sem_g, "a": sem_a}

            @block.sync
            def _(sync):
                last = {}
                dlast = {}
                for op_eng, emit_fn, deps, ref in sch.ops:
                    if op_eng != "s":
                        continue
                    for dep in deps:
                        if dep[0] == "D":
                            _, slot, k = dep
                            if dlast.get(slot, 0) >= k:
                                continue
                            sync.wait_ge(dma_sems[slot], 16 * k)
                            dlast[slot] = k
                        else:
                            deng, dpos = dep
                            if deng == "s" or last.get(deng, 0) >= dpos:
                                continue
                            sync.wait_ge(sems[deng], dpos)
                            last[deng] = dpos
                    emit_fn().then_inc(dma_sems[ref[1]], 16)

            @block.vector
            def _(vector):
                sch.emit("v", vector, sems, dma_sems)

            @block.gpsimd
            def _(gp):
                sch.emit("g", gp, sems, dma_sems)

            @block.scalar
            def _(scalar):
                sch.emit("a", scalar, sems, dma_sems)

    return nc


_cache = {}


def _get_nc(reps=1):
    if reps not in _cache:
        _cache[reps] = _build_nc(reps)
    return _cache[reps]


def _pack_core(z_core, mu_core, times_core):
    arr = np.concatenate([
        np.ascontiguousarray(z_core.transpose(2, 1, 0)),      # (5, NPC, T)
        np.ascontiguousarray(times_core.T)[None],             # (1, NPC, T)
    ], axis=0)
    IN = np.ascontiguousarray(
        arr.reshape(6, NSL, P, CH, T).transpose(1, 0, 2, 3, 4))
    MU = np.ascontiguousarray(
        mu_core.T.reshape(5, NSL, P, CH).transpose(1, 0, 2, 3))
    return {"inp": IN, "mu": MU}


def _make_in_maps(z_and_L_hat, mu0, times):
    z_and_L_hat = np.asarray(z_and_L_hat, dtype=np.float32)
    mu0 = np.asarray(mu0, dtype=np.float32)
    times = np.asarray(times, dtype=np.float32)
    in_maps = []
    for k in range(N_CORES):
        sl = slice(k * NPC, (k + 1) * NPC)
        in_maps.append(_pack_core(z_and_L_hat[:, sl, :], mu0[sl], times[:, sl]))
    return in_maps


def kernel(z_and_L_hat, mu0, times):
    nc = _get_nc()
    in_maps = _make_in_maps(z_and_L_hat, mu0, times)
    res = run_bass_kernel_spmd(nc, in_maps, core_ids=list(range(N_CORES)))
    out = np.empty((T, N_TOT, 3), np.float32)
    for k in range(N_CORES):
        O = res.results[k]["out"]                 # (NSL, 3, P, CH, T)
        planes = O.transpose(1, 0, 2, 3, 4).reshape(3, NPC, T)
        sl = slice(k * NPC, (k + 1) * NPC)
        out[:, sl, 0] = planes[0].T
        out[:, sl, 1] = planes[1].T
        out[:, sl, 2] = planes[2].T
    return out

